# revision 22
# baseline (speedup 1.0000x reference)
"""Trainium2 Bass kernel for the 2-hop key-value memory network.

Strategy: data-parallel over batch (B=32 -> 4 per core x 8 cores).

The gather stream (48 x 1024 rows/core from a per-core dense unique-token
table) is DMA-bandwidth-bound (~360 GB/s/core); everything else hides under
it.  Key structure:

  - Rows are [A1 | A2 | A0] with the A0 part (hop-0 score embeddings) stored
    as fp8e4 (x64 scale, 1/64 folded into its word-sum sel matrix): hop-0
    outputs are a ~2% perturbation of the persistent state so the fp8 noise
    is damped ~50x.  Row = 1280B instead of 1536B -> ~17% less DMA.
  - Per-batch hop math is restructured so the serial tail after the last
    gathers is minimal: scores are computed per chunk, and the weighted
    sums use  O = (R - A*C) / B  with  R = S^T @ E  and C = colsum(E)
    (replicated across the 9 score rows by an all-ones lhsT so the final
    affine is pure DVE) accumulated per chunk DURING the gathers.  No
    softmax-weight materialization / transposes / matmuls on the tail, and
    no tail PE op feeds from the serial renorm chain except via DVE.
  - 4 SWDGE queues for gather descriptor-generation concurrency.

Renorms reduce to per-row affine transforms:
  renorm_q: p = (s - min(s)) / (sum(s) - 512*min(s) + 512e-8)
  renorm_a: p = (s - lse)   / (512*lse - sum(s)),  lse = logsumexp(s)
"""
import sys

for _p in ("/opt/pypackages", "/opt/trn_rl_repo"):
    if _p not in sys.path:
        sys.path.insert(0, _p)

import numpy as np
import ml_dtypes

import concourse.bass as bass
import concourse.bacc as bacc
import concourse.mybir as mybir
import concourse.tile as tile
from concourse.bass_utils import run_bass_kernel_spmd

BF = ml_dtypes.bfloat16
F8 = ml_dtypes.float8_e4m3

# problem constants
B, NMEM, NW, QLEN, NCH, CLEN = 32, 512, 8, 32, 8, 8
VOCAB, M = 50000, 256
EMB = 3 * M           # 768
NCORES = 8
BL = B // NCORES      # 4 batch per core
NCHUNK = 4            # mem chunks of 128 per (b, tau)
BU = 384              # b-table unique slots per core

A0FP8 = False         # fp8 matmul rhs streams ~3x slower than bf16 on PE,
                      # and the stream is desc-gen-bound, so fp8 rows lose
A0SCALE = 64.0
ROWE = 640 if A0FP8 else 768   # bf16 elements per dense-table row

_cache = {}


def _wrap_idx16(stream):
    """stream (len multiple of 16) -> [128, len/16] int16 wrapped layout:
    index i lives at [i % 16, i // 16], replicated across the 8 groups of 16
    partitions."""
    c = len(stream) // 16
    arr = stream.astype(np.uint16).reshape(c, 16).T
    return np.tile(arr, (8, 1)).view(np.int16)


def _build_program(nu_pad):
    """Build + compile the SPMD program (same for all cores)."""
    if nu_pad in _cache:
        return _cache[nu_pad]

    f32 = mybir.dt.float32
    bf16 = mybir.dt.bfloat16
    f8e4 = mybir.dt.float8e4

    nc = bacc.Bacc("TRN2", target_bir_lowering=False, debug=False,
                   num_swdge_queues=4,
                   dynamic_dma_scratch_size=24576)
    tab_d = nc.dram_tensor("tab", [nu_pad, ROWE], bf16, kind="ExternalInput")
    btab_d = nc.dram_tensor("btab", [BU, EMB], bf16, kind="ExternalInput")
    ut_d = nc.dram_tensor("ut", [EMB, EMB], bf16, kind="ExternalInput")
    vt_d = nc.dram_tensor("vt", [EMB, EMB], bf16, kind="ExternalInput")
    w_d = nc.dram_tensor("w", [EMB, EMB], bf16, kind="ExternalInput")
    idx16_d = nc.dram_tensor("idx16", [128, 3 * BL, NCHUNK * 64],
                             mybir.dt.int16, kind="ExternalInput")
    selb_d = nc.dram_tensor("selb", [128, 8 * 128], bf16,
                            kind="ExternalInput")
    self8_d = nc.dram_tensor("self8", [128, 8 * 128], mybir.dt.uint8,
                             kind="ExternalInput")
    ones3_d = nc.dram_tensor("ones3", [3, 128, 128], bf16,
                             kind="ExternalInput")
    maskq_d = nc.dram_tensor("maskq", [128, 1], f32, kind="ExternalInput")
    identb_d = nc.dram_tensor("identb", [128, 128], bf16,
                              kind="ExternalInput")
    crepsel_d = nc.dram_tensor("crepsel", [128, 9 * BL], bf16,
                               kind="ExternalInput")
    idxua_d = nc.dram_tensor("idxua", [128, 3], mybir.dt.int32,
                             kind="ExternalInput")
    out_d = nc.dram_tensor("pred", [BL, NCH], f32, kind="ExternalOutput")

    with tile.TileContext(nc) as tc:
        with (
            tc.tile_pool(name="const", bufs=1) as constp,
            tc.tile_pool(name="state", bufs=1) as statep,
            tc.tile_pool(name="gp", bufs=7 if A0FP8 else 6) as gp,
            tc.tile_pool(name="epA", bufs=1) as epA,
            tc.tile_pool(name="epB", bufs=2) as epB,
            tc.tile_pool(name="stgp", bufs=2) as stgp,
            tc.tile_pool(name="hsb", bufs=1) as hsb,
            tc.tile_pool(name="sb2", bufs=2) as sb2,
            tc.tile_pool(name="osb", bufs=1) as osbp,
            tc.tile_pool(name="wsps", bufs=1, space="PSUM") as wsps,
            tc.tile_pool(name="tpp", bufs=2, space="PSUM") as tpp,
            tc.tile_pool(name="scp", bufs=1, space="PSUM") as scp,
            tc.tile_pool(name="rcp", bufs=1, space="PSUM") as rcp,
            tc.tile_pool(name="auxp", bufs=1, space="PSUM") as auxp,
        ):
            # ---------------- constants ----------------
            idx_sb = constp.tile([128, 3 * BL, NCHUNK * 64], mybir.dt.int16,
                                 tag="idx")
            # first batch's slice lands fast so gather #0 starts early
            nc.sync.dma_start(out=idx_sb[:, 0:3, :], in_=idx16_d[:, 0:3, :])
            nc.sync.dma_start(out=idx_sb[:, 3:3 * BL, :],
                              in_=idx16_d[:, 3:3 * BL, :])
            selb = constp.tile([128, 8 * 128], bf16, tag="selb")
            nc.sync.dma_start(out=selb[:], in_=selb_d[:])
            self8 = constp.tile([128, 8 * 128], mybir.dt.uint8, tag="self8")
            nc.sync.dma_start(out=self8[:], in_=self8_d[:])
            identb = constp.tile([128, 128], bf16, tag="identb")
            nc.sync.dma_start(out=identb[:], in_=identb_d[:])
            crepsel = constp.tile([128, 9 * BL], bf16, tag="crepsel")
            nc.sync.dma_start(out=crepsel[:], in_=crepsel_d[:])
            maskq = constp.tile([128, 1], f32, tag="maskq")
            nc.sync.dma_start(out=maskq[:], in_=maskq_d[:])
            ones3 = [constp.tile([128, 128], bf16, tag=f"ones{i}",
                                 name=f"ones{i}") for i in range(3)]
            for i in range(3):
                nc.sync.dma_start(out=ones3[i][:], in_=ones3_d[i])
            idxua = constp.tile([128, 3], mybir.dt.int32, tag="idxua")
            nc.sync.dma_start(out=idxua[:], in_=idxua_d[:])
            ut_sb = constp.tile([128, 6, EMB], bf16, tag="ut")
            vt_sb = constp.tile([128, 6, EMB], bf16, tag="vt")
            w_sb = constp.tile([128, 6, EMB], bf16, tag="w")

            def emit_uvw_dma():
                for t_sb, t_d in ((ut_sb, ut_d), (vt_sb, vt_d),
                                  (w_sb, w_d)):
                    nc.sync.dma_start(
                        out=t_sb[:],
                        in_=t_d[:].rearrange("(j p) d -> p j d", p=128))

            # persistent state
            X = statep.tile([128, EMB], f32, tag="X")
            o_sb = statep.tile([128, EMB], bf16, tag="o_sb")
            xtq = statep.tile([128, 6, BL], bf16, tag="xtq")
            xta = statep.tile([128, 6, 8 * BL], bf16, tag="xta")
            ysb0 = statep.tile([128, 6, 9 * BL], bf16, tag="ysb0")
            pred_sb = statep.tile([128, NCH], f32, tag="pred_sb")
            gb = statep.tile([128, 3, EMB], bf16, tag="gb")
            # zeroed once: rows outside the written 10-row band must be
            # finite for the crepsel rank-1 matmul (0 * garbage != 0 if NaN)
            RSBbf = statep.tile([128, EMB], bf16, tag="rsbbf")
            nc.vector.memset(RSBbf[:], 0)

            # warm-up: force the SWDGE gather ucode LOAD_LIB before the
            # idx DMA lands so the first real gather isn't serialized on it
            warm_idx = constp.tile([128, 1], mybir.dt.int16, tag="warm_idx")
            nc.vector.memset(warm_idx[:], 0)
            warm_g = constp.tile([128, 1, EMB], bf16, tag="warm_g")
            nc.gpsimd.dma_gather(warm_g[:], btab_d[:], warm_idx[:],
                                 16, 16, EMB)

            # ---------------- init: u and a from B_table ----------------
            def emit_init():
                for i in range(3):
                    nc.gpsimd.indirect_dma_start(
                        out=gb[:, i, :], out_offset=None, in_=btab_d[:],
                        in_offset=bass.IndirectOffsetOnAxis(
                            ap=idxua[:, i:i + 1], axis=0))
                for lo, hi in ((0, 512), (512, 768)):
                    psi = auxp.tile([128, 512], f32, tag="aux", name="psi")
                    for i in range(3):
                        nc.tensor.matmul(out=psi[:, 0:hi - lo],
                                         lhsT=ones3[i][:],
                                         rhs=gb[:, i, lo:hi],
                                         start=(i == 0), stop=(i == 2))
                    nc.vector.tensor_copy(out=X[:, lo:hi],
                                          in_=psi[:, 0:hi - lo])
                Xbf = sb2.tile([128, EMB], bf16, tag="Xbf", name="Xbf0")
                nc.any.tensor_copy(out=Xbf[:], in_=X[:])
                for j in range(6):
                    tp = tpp.tile([128, 128], bf16, tag="tp", name="tpi")
                    nc.tensor.transpose(out=tp[:],
                                        in_=Xbf[:, 128 * j:128 * j + 128],
                                        identity=identb[:])
                    tpv = tp[:].rearrange("p (b n) -> p b n", b=BL)
                    nc.any.tensor_copy(out=xtq[:, j, :], in_=tpv[:, :, 0])
                    nc.any.tensor_copy(out=xta[:, j, :], in_=tpv[:, :, 1:9])
                for i in range(6):
                    y0 = auxp.tile([128, 512], f32, tag="aux", name="y0")
                    for j in range(6):
                        nc.tensor.matmul(
                            out=y0[:, 0:BL],
                            lhsT=ut_sb[:, j, 128 * i:128 * i + 128],
                            rhs=xtq[:, j, :], start=(j == 0), stop=(j == 5))
                    for j in range(6):
                        nc.tensor.matmul(
                            out=y0[:, BL:9 * BL],
                            lhsT=vt_sb[:, j, 128 * i:128 * i + 128],
                            rhs=xta[:, j, :], start=(j == 0), stop=(j == 5))
                    y9 = ysb0[:, i, :].rearrange("p (b r) -> p b r", b=BL)
                    ya4 = y0[:, BL:9 * BL].rearrange("p (b r) -> p b r", b=BL)
                    nc.any.tensor_copy(out=y9[:, :, 0], in_=y0[:, 0:BL])
                    nc.any.tensor_copy(out=y9[:, :, 1:9], in_=ya4[:])

            # ---------------- renorm constants ----------------
            def renorm_part1(b, prt, Ssb, hop):
                """Final reduces + exp-accum (DVE+scalar, no PE)."""
                rows = slice(32 * b, 32 * b + 9)
                t = lambda tag: hsb.tile([128, 1], f32, tag=tag, name=tag)
                mx, mn, sm = t("mx"), t("mn"), t("sm")
                nc.vector.tensor_reduce(out=mx[rows], in_=prt[rows, 0:4],
                                        axis=mybir.AxisListType.X,
                                        op=mybir.AluOpType.max)
                nc.vector.tensor_reduce(out=mn[rows], in_=prt[rows, 4:8],
                                        axis=mybir.AxisListType.X,
                                        op=mybir.AluOpType.min)
                nc.vector.tensor_reduce(out=sm[rows], in_=prt[rows, 8:12],
                                        axis=mybir.AxisListType.X,
                                        op=mybir.AluOpType.add)
                negmx = t("negmx")
                nc.vector.tensor_scalar(out=negmx[rows], in0=mx[rows],
                                        scalar1=-1.0, scalar2=None,
                                        op0=mybir.AluOpType.mult)
                texp = sb2.tile([128, NMEM], f32, tag="texp", name="texp")
                se = t("se")
                nc.scalar.activation(out=texp[rows], in_=Ssb[rows, 0:NMEM],
                                     func=mybir.ActivationFunctionType.Exp,
                                     bias=negmx[rows], scale=1.0,
                                     accum_out=se[rows])
                return mx, mn, sm, se

            def renorm_part2(b, mx, mn, sm, se, hop):
                """ln(se) via DVE bit-trick cubic (avoids the Exp<->Ln
                activation-table thrash) + affine-constant blends -> A, 1/B."""
                rows = slice(32 * b, 32 * b + 9)
                t = lambda tag: hsb.tile([128, 1], f32, tag=tag, name=tag)
                # se = m * 2^e, m in [1,2);  ln(se) = ln2*(u>>23) + poly(m)
                # with -127*ln2 folded into the poly's constant term
                LN2 = 0.6931471805599453
                C3, C2, C1 = 0.10668396, -0.71358544, 2.08687084
                C0 = -1.47904405 - 127.0 * LN2
                sei = se[rows].bitcast(mybir.dt.int32)
                mb = hsb.tile([128, 1], mybir.dt.int32, tag="mb", name="mb")
                nc.vector.tensor_scalar(out=mb[rows], in0=sei,
                                        scalar1=0x007FFFFF,
                                        scalar2=0x3F800000,
                                        op0=mybir.AluOpType.bitwise_and,
                                        op1=mybir.AluOpType.bitwise_or)
                mf = mb[rows].bitcast(f32)
                efi = hsb.tile([128, 1], mybir.dt.int32, tag="efi",
                               name="efi")
                nc.vector.tensor_scalar(out=efi[rows], in0=sei,
                                        scalar1=23, scalar2=None,
                                        op0=mybir.AluOpType.logical_shift_right)
                ef = t("ef")
                nc.vector.tensor_scalar(out=ef[rows], in0=efi[rows],
                                        scalar1=LN2, scalar2=None,
                                        op0=mybir.AluOpType.mult)
                lse = t("lse")
                nc.vector.tensor_scalar(out=lse[rows], in0=mf,
                                        scalar1=C3, scalar2=C2,
                                        op0=mybir.AluOpType.mult,
                                        op1=mybir.AluOpType.add)
                nc.vector.tensor_tensor(out=lse[rows], in0=lse[rows],
                                        in1=mf, op=mybir.AluOpType.mult)
                nc.vector.tensor_scalar(out=lse[rows], in0=lse[rows],
                                        scalar1=C1, scalar2=None,
                                        op0=mybir.AluOpType.add)
                nc.vector.tensor_tensor(out=lse[rows], in0=lse[rows],
                                        in1=mf, op=mybir.AluOpType.mult)
                nc.vector.tensor_scalar(out=lse[rows], in0=lse[rows],
                                        scalar1=C0, scalar2=None,
                                        op0=mybir.AluOpType.add)
                nc.vector.tensor_tensor(out=lse[rows], in0=lse[rows],
                                        in1=ef[rows],
                                        op=mybir.AluOpType.add)
                nc.vector.tensor_tensor(out=lse[rows], in0=lse[rows],
                                        in1=mx[rows],
                                        op=mybir.AluOpType.add)
                t1 = t("t1")
                nc.vector.tensor_tensor(out=t1[rows], in0=mn[rows],
                                        in1=lse[rows],
                                        op=mybir.AluOpType.subtract)
                nc.vector.tensor_tensor(out=t1[rows], in0=t1[rows],
                                        in1=maskq[rows],
                                        op=mybir.AluOpType.mult)
                Av = t(f"Av{hop}")
                nc.vector.tensor_tensor(out=Av[rows], in0=lse[rows],
                                        in1=t1[rows],
                                        op=mybir.AluOpType.add)
                bq = t("bq")
                nc.vector.tensor_scalar(out=bq[rows], in0=mn[rows],
                                        scalar1=-512.0, scalar2=512e-8,
                                        op0=mybir.AluOpType.mult,
                                        op1=mybir.AluOpType.add)
                nc.vector.tensor_tensor(out=bq[rows], in0=bq[rows],
                                        in1=sm[rows],
                                        op=mybir.AluOpType.add)
                ba = t("ba")
                nc.vector.tensor_scalar(out=ba[rows], in0=lse[rows],
                                        scalar1=512.0, scalar2=None,
                                        op0=mybir.AluOpType.mult)
                nc.vector.tensor_tensor(out=ba[rows], in0=ba[rows],
                                        in1=sm[rows],
                                        op=mybir.AluOpType.subtract)
                Bv = t("Bv")
                nc.vector.tensor_tensor(out=Bv[rows], in0=bq[rows],
                                        in1=ba[rows],
                                        op=mybir.AluOpType.subtract)
                nc.vector.tensor_tensor(out=Bv[rows], in0=Bv[rows],
                                        in1=maskq[rows],
                                        op=mybir.AluOpType.mult)
                nc.vector.tensor_tensor(out=Bv[rows], in0=Bv[rows],
                                        in1=ba[rows],
                                        op=mybir.AluOpType.add)
                invb = t(f"invb{hop}")
                nc.vector.reciprocal(out=invb[rows], in_=Bv[rows])
                return Av, invb

            def affine_O(b, RSB, Crep, Av, invb, out_tile, add_to_X):
                """out = (R - A*Crep)/B, optionally X += out.  Pure DVE."""
                rows = slice(32 * b, 32 * b + 9)
                tmp = osbp.tile([128, EMB], f32, tag="otmp", name="otmp")
                nc.vector.tensor_scalar(out=tmp[rows, :],
                                        in0=Crep[rows, :],
                                        scalar1=Av[rows], scalar2=None,
                                        op0=mybir.AluOpType.mult)
                nc.vector.tensor_tensor(out=tmp[rows, :], in0=RSB[rows, :],
                                        in1=tmp[rows, :],
                                        op=mybir.AluOpType.subtract)
                nc.vector.tensor_scalar(out=out_tile[rows, :],
                                        in0=tmp[rows, :],
                                        scalar1=invb[rows], scalar2=None,
                                        op0=mybir.AluOpType.mult)
                if add_to_X:
                    nc.vector.tensor_tensor(out=X[rows, :], in0=X[rows, :],
                                            in1=out_tile[rows, :],
                                            op=mybir.AluOpType.add)

            def score_chunk(b, c, ysrc, ET, Eb, Ssb, prt, stage, RSB):
                """Per-chunk: scores, partial reduces, S^T staging, R acc.
                stage col 9 is preset to 1.0 so R row 32b+9 accumulates
                C = colsum(E)."""
                rows = slice(32 * b, 32 * b + 9)
                rows10 = slice(32 * b, 32 * b + 10)
                mc = slice(128 * c, 128 * c + 128)
                Sc = scp.tile([128, 128], f32, tag="sc", name="Sc")
                for j in range(6):
                    nc.tensor.matmul(out=Sc[rows, :], lhsT=ysrc(j),
                                     rhs=ET[:, j, mc],
                                     start=(j == 0), stop=(j == 5),
                                     tile_position=(0, 32 * b))
                nc.any.tensor_copy(out=Ssb[rows, mc], in_=Sc[rows, :])
                nc.vector.tensor_reduce(out=prt[rows, c:c + 1],
                                        in_=Sc[rows, :],
                                        axis=mybir.AxisListType.X,
                                        op=mybir.AluOpType.max)
                nc.vector.tensor_reduce(out=prt[rows, 4 + c:5 + c],
                                        in_=Sc[rows, :],
                                        axis=mybir.AxisListType.X,
                                        op=mybir.AluOpType.min)
                nc.vector.tensor_reduce(out=prt[rows, 8 + c:9 + c],
                                        in_=Sc[rows, :],
                                        axis=mybir.AxisListType.X,
                                        op=mybir.AluOpType.add)
                tp = tpp.tile([128, 128], bf16, tag="tp", name="tpS")
                nc.tensor.transpose(out=tp[:], in_=Ssb[:, mc],
                                    identity=identb[:])
                nc.any.tensor_copy(out=stage[:, c, 0:9],
                                   in_=tp[:, 32 * b:32 * b + 9])
                Rc = rcp.tile([128, EMB], f32, tag="rc", name="Rc")
                nc.tensor.matmul(out=Rc[rows10, 0:512],
                                 lhsT=stage[:, c, 0:10],
                                 rhs=Eb[:, c, 0:512], start=True, stop=True,
                                 tile_position=(0, 32 * b))
                nc.tensor.matmul(out=Rc[rows10, 512:768],
                                 lhsT=stage[:, c, 0:10],
                                 rhs=Eb[:, c, 512:768], start=True,
                                 stop=True, tile_position=(0, 32 * b))
                if c == 0:
                    nc.vector.tensor_copy(out=RSB[rows10, :],
                                          in_=Rc[rows10, :])
                else:
                    nc.vector.tensor_tensor(out=RSB[rows10, :],
                                            in0=RSB[rows10, :],
                                            in1=Rc[rows10, :],
                                            op=mybir.AluOpType.add)

            def emit_crep(b, RSB):
                """CrepP[32b+t, :] = RSB row 32b+9 (= colsum E) for t<9,
                via a rank-1 selector matmul; runs parallel to the renorm."""
                rows10 = slice(32 * b, 32 * b + 10)
                nc.any.tensor_copy(out=RSBbf[rows10, :], in_=RSB[rows10, :])
                CrepP = rcp.tile([128, EMB], f32, tag="rc", name="CrepP")
                nc.tensor.matmul(out=CrepP[32 * b:32 * b + 9, 0:512],
                                 lhsT=crepsel[:, 9 * b:9 * b + 9],
                                 rhs=RSBbf[:, 0:512], start=True, stop=True,
                                 tile_position=(0, 32 * b))
                nc.tensor.matmul(out=CrepP[32 * b:32 * b + 9, 512:768],
                                 lhsT=crepsel[:, 9 * b:9 * b + 9],
                                 rhs=RSBbf[:, 512:768], start=True,
                                 stop=True, tile_position=(0, 32 * b))
                return CrepP

            # ---------------- per-batch tail ----------------
            def hop_tail(b, E1T, E2b, RSB0, Ssb0, prt0):
                rows = slice(32 * b, 32 * b + 9)
                # seg 1a: Crep (PE, parallel to renorm) + renorm0 reduces
                Crep0 = emit_crep(b, RSB0)
                r0 = renorm_part1(b, prt0, Ssb0, 0)
                yield
                # seg 1b: renorm0 constants
                Av0, invb0 = renorm_part2(b, *r0, 0)
                yield
                # seg 2a: O0 affine + X update + Xbf cast (pure DVE)
                O0sb = osbp.tile([128, EMB], f32, tag="O0sb", name="O0sb")
                affine_O(b, RSB0, Crep0, Av0, invb0, O0sb, add_to_X=True)
                Xbf = sb2.tile([128, EMB], bf16, tag="Xbf", name="Xbf")
                nc.any.tensor_copy(out=Xbf[rows, :], in_=X[rows, :])
                yield
                # seg 2b: xt1 staging transposes
                xt1u = hsb.tile([128, 6, 1], bf16, tag=f"xt1u_{b}",
                                name="xt1u")
                xta1 = hsb.tile([128, 6, 9], bf16, tag=f"xta1_{b}",
                                name="xta1")
                nc.vector.memset(xta1[:, :, 0:1], 0)
                for j in range(6):
                    tp = tpp.tile([128, 128], bf16, tag="tp", name="tpX")
                    nc.tensor.transpose(out=tp[:],
                                        in_=Xbf[:, 128 * j:128 * j + 128],
                                        identity=identb[:])
                    nc.any.tensor_copy(out=xt1u[:, j, :],
                                       in_=tp[:, 32 * b:32 * b + 1])
                    nc.any.tensor_copy(out=xta1[:, j, 1:9],
                                       in_=tp[:, 32 * b + 1:32 * b + 9])
                yield
                # seg 3: y1^T (two column-halves), ysb1
                y1Tsb = sb2.tile([128, EMB], bf16, tag="y1Tsb", name="y1Tsb")
                for lo, hi in ((0, 512), (512, 768)):
                    y1T = auxp.tile([128, 512], f32, tag="aux", name="y1T")
                    for j in range(6):
                        nc.tensor.matmul(out=y1T[0:9, 0:hi - lo],
                                         lhsT=xta1[:, j, 0:9],
                                         rhs=vt_sb[:, j, lo:hi],
                                         start=(j == 0), stop=False)
                    for j in range(6):
                        nc.tensor.matmul(out=y1T[0:1, 0:hi - lo],
                                         lhsT=xt1u[:, j, 0:1],
                                         rhs=ut_sb[:, j, lo:hi],
                                         start=False, stop=(j == 5))
                    nc.any.tensor_copy(out=y1Tsb[0:9, lo:hi],
                                       in_=y1T[0:9, 0:hi - lo])
                ysb1 = hsb.tile([128, 6, 9], bf16, tag=f"ysb1_{b}",
                                name="ysb1")
                for j in range(6):
                    tp = tpp.tile([128, 128], bf16, tag="tp", name="tpY")
                    nc.tensor.transpose(out=tp[:],
                                        in_=y1Tsb[:, 128 * j:128 * j + 128],
                                        identity=identb[:])
                    nc.any.tensor_copy(out=ysb1[:, j, :], in_=tp[:, 0:9])
                yield
                # seg 4: hop1 scores per chunk + R1
                RSB1 = sb2.tile([128, EMB], f32, tag="rsb1", name="RSB1")
                Ssb1 = sb2.tile([128, NMEM], bf16, tag="ssb1", name="Ssb1")
                prt1 = hsb.tile([128, 12], f32, tag=f"prt1_{b}", name="prt1")
                stage1 = hsb.tile([128, NCHUNK, 16], bf16,
                                  tag=f"stage1_{b}", name="stage1")
                nc.vector.memset(stage1[:, :, 9:10], 1.0)
                for c in range(2):
                    score_chunk(b, c, lambda j: ysb1[:, j, 0:9], E1T, E2b,
                                Ssb1, prt1, stage1, RSB1)
                yield
                for c in range(2, NCHUNK):
                    score_chunk(b, c, lambda j: ysb1[:, j, 0:9], E1T, E2b,
                                Ssb1, prt1, stage1, RSB1)
                yield
                # seg 5a/5b: Crep1 (PE) + renorm1 (DVE/scalar)
                Crep1 = emit_crep(b, RSB1)
                r1 = renorm_part1(b, prt1, Ssb1, 1)
                yield
                Av1, invb1 = renorm_part2(b, *r1, 1)
                yield
                # seg 6a: O1 affine -> o_sb (bf16 via f32 tmp + copy)
                O1sb = osbp.tile([128, EMB], f32, tag="O0sb", name="O1sb")
                affine_O(b, RSB1, Crep1, Av1, invb1, O1sb, add_to_X=False)
                nc.any.tensor_copy(out=o_sb[rows, :], in_=O1sb[rows, :])
                yield
                # seg 6b: final bilinear form
                ot = hsb.tile([128, 6, 16], bf16, tag=f"ot_{b}", name="ot")
                for j in range(6):
                    tp = tpp.tile([128, 128], bf16, tag="tp", name="tpO")
                    nc.tensor.transpose(
                        out=tp[:], in_=o_sb[:, 128 * j:128 * j + 128],
                        identity=identb[:])
                    nc.any.tensor_copy(out=ot[:, j, 0:9],
                                       in_=tp[:, 32 * b:32 * b + 9])
                tsb = sb2.tile([128, EMB], bf16, tag="tsb", name="tsb")
                for lo, hi in ((0, 512), (512, 768)):
                    tps = auxp.tile([128, 512], f32, tag="aux", name="tvec")
                    for j in range(6):
                        nc.tensor.matmul(out=tps[0:1, 0:hi - lo],
                                         lhsT=ot[:, j, 0:1],
                                         rhs=w_sb[:, j, lo:hi],
                                         start=(j == 0), stop=(j == 5))
                    nc.any.tensor_copy(out=tsb[0:1, lo:hi],
                                       in_=tps[0:1, 0:hi - lo])
                tT = hsb.tile([128, 6, 1], bf16, tag=f"tT_{b}", name="tT")
                for j in range(6):
                    tp = tpp.tile([128, 128], bf16, tag="tp", name="tpT")
                    nc.tensor.transpose(out=tp[:],
                                        in_=tsb[:, 128 * j:128 * j + 128],
                                        identity=identb[:])
                    nc.any.tensor_copy(out=tT[:, j, :], in_=tp[:, 0:1])
                predp = scp.tile([128, 128], f32, tag="sc", name="predp")
                for j in range(6):
                    nc.tensor.matmul(
                        out=predp[32 * b:32 * b + 1, 0:NCH],
                        lhsT=tT[:, j, 0:1], rhs=ot[:, j, 1:9],
                        start=(j == 0), stop=(j == 5),
                        tile_position=(0, 32 * b))
                nc.vector.tensor_copy(out=pred_sb[32 * b:32 * b + 1, :],
                                      in_=predp[32 * b:32 * b + 1, 0:NCH])
                nc.sync.dma_start(out=out_d[b:b + 1, :],
                                  in_=pred_sb[32 * b:32 * b + 1, :])
                yield

            # ---------------- main loop ----------------
            pending = None
            for b in range(BL):
                E0T = epA.tile([128, 6, NMEM], bf16, tag="E0T", name="E0T")
                E1b = epA.tile([128, NCHUNK, EMB], bf16, tag="E1b",
                               name="E1b")
                E1T = epB.tile([128, 6, NMEM], bf16, tag="E1T", name="E1T")
                E2b = epB.tile([128, NCHUNK, EMB], bf16, tag="E2b",
                               name="E2b")
                RSB0 = sb2.tile([128, EMB], f32, tag="rsb0", name="RSB0")
                Ssb0 = sb2.tile([128, NMEM], bf16, tag="ssb0", name="Ssb0")
                prt0 = hsb.tile([128, 12], f32, tag=f"prt0_{b}", name="prt0")
                stage0 = hsb.tile([128, NCHUNK, 16], bf16,
                                  tag=f"stage0_{b}", name="stage0")
                nc.vector.memset(stage0[:, :, 9:10], 1.0)
                for c in range(NCHUNK):
                    for tau in range(3):
                        tb = b * 3 + tau
                        g = gp.tile([128, 8, ROWE], bf16, tag="g", name="g")
                        nc.gpsimd.dma_gather(
                            g[:], tab_d[:],
                            idx_sb[:, tb, 64 * c:64 * c + 64],
                            1024, 1024, ROWE,
                            queue_num=(c * 3 + tau) % 4)
                        ps_ab = wsps.tile([128, 512], f32, tag="wsab",
                                          name="ps_ab")
                        ps_e0 = wsps.tile([128, 256], f32, tag="wse0",
                                          name="ps_e0")
                        for gi in range(8):
                            k = gi // 2
                            lsl = slice(128 * gi + 32 * k,
                                        128 * gi + 32 * k + 32)
                            lhs = selb[:, lsl]
                            nc.tensor.matmul(
                                out=ps_ab[32 * k:32 * k + 32, :], lhsT=lhs,
                                rhs=g[:, gi, 0:512],
                                start=(gi % 2 == 0), stop=(gi % 2 == 1),
                                tile_position=(0, 32 * k))
                            if A0FP8:
                                rhs0 = g[:, gi, 512:640].bitcast(f8e4)
                                lhs0 = self8[:, lsl].bitcast(f8e4)
                            else:
                                rhs0 = g[:, gi, 512:768]
                                lhs0 = lhs
                            nc.tensor.matmul(
                                out=ps_e0[32 * k:32 * k + 32, :], lhsT=lhs0,
                                rhs=rhs0,
                                start=(gi % 2 == 0), stop=(gi % 2 == 1),
                                tile_position=(0, 32 * k))
                        dl = 256 * tau
                        nc.any.tensor_copy(out=E1b[:, c, dl:dl + 256],
                                           in_=ps_ab[:, 0:256])
                        nc.any.tensor_copy(out=E2b[:, c, dl:dl + 256],
                                           in_=ps_ab[:, 256:512])
                        stg = stgp.tile([128, 256], bf16, tag="stg",
                                        name="stg")
                        nc.any.tensor_copy(out=stg[:], in_=ps_e0[:])
                        for q in range(2):   # E1T from E1b directly
                            tp = tpp.tile([128, 128], bf16, tag="tp",
                                          name="tp")
                            nc.tensor.transpose(
                                out=tp[:],
                                in_=E1b[:, c, dl + 128 * q:dl + 128 * q
                                        + 128],
                                identity=identb[:])
                            nc.any.tensor_copy(
                                out=E1T[:, 2 * tau + q,
                                        128 * c:128 * c + 128],
                                in_=tp[:])
                        for q in range(2):   # E0T from stg
                            tp = tpp.tile([128, 128], bf16, tag="tp",
                                          name="tp")
                            nc.tensor.transpose(
                                out=tp[:], in_=stg[:, 128 * q:128 * q + 128],
                                identity=identb[:])
                            nc.any.tensor_copy(
                                out=E0T[:, 2 * tau + q,
                                        128 * c:128 * c + 128],
                                in_=tp[:])
                        if pending is not None:
                            next(pending, None)
                    if b == 0 and c == 0:
                        emit_uvw_dma()
                        emit_init()
                    score_chunk(b, c, lambda j: ysb0[:, j, 9 * b:9 * b + 9],
                                E0T, E1b, Ssb0, prt0, stage0, RSB0)
                    if pending is not None:
                        next(pending, None)
                if pending is not None:
                    for _ in pending:
                        pass
                pending = hop_tail(b, E1T, E2b, RSB0, Ssb0, prt0)
            for _ in pending:
                pass

    nc.compile()
    _cache[nu_pad] = nc
    return nc


def _prepare(subjects, relations, objects, ques, answerChoices,
             A_tables, B_table, U, V, W):
    subjects = np.asarray(subjects).astype(np.int64)
    relations = np.asarray(relations).astype(np.int64)
    objects = np.asarray(objects).astype(np.int64)
    ques = np.asarray(ques).astype(np.int64)
    answerChoices = np.asarray(answerChoices).astype(np.int64)
    A_tables = np.asarray(A_tables, dtype=np.float32)
    B_table = np.asarray(B_table, dtype=np.float32)

    b_bf = B_table.astype(BF)
    ut = np.ascontiguousarray(np.asarray(U, dtype=np.float32).T).astype(BF)
    vt = np.ascontiguousarray(np.asarray(V, dtype=np.float32).T).astype(BF)
    w_bf = np.ascontiguousarray(np.asarray(W, dtype=np.float32)).astype(BF)
    identb = np.eye(128, dtype=BF)
    crepsel = np.zeros((128, 9 * BL), dtype=BF)
    for b in range(BL):
        crepsel[32 * b + 9, 9 * b:9 * b + 9] = 1.0
    maskq = np.zeros((128, 1), dtype=np.float32)
    maskq[0::32] = 1.0
    # fixed word-sum selection: slot s = gi*128+p -> mem 16*gi + p//8
    p = np.arange(128)
    selb = np.zeros((128, 8, 128), dtype=BF)
    for gi in range(8):
        selb[p, gi, 16 * gi + p // 8] = 1.0
    self8 = np.zeros((128, 8, 128), dtype=F8)
    for gi in range(8):
        self8[p, gi, 16 * gi + p // 8] = F8(1.0 / A0SCALE if A0FP8 else 1.0)
    selb = selb.reshape(128, 8 * 128)
    self8 = self8.reshape(128, 8 * 128).view(np.uint8)
    # init placement matrices (state row = 32*b + tc)
    ones3 = np.zeros((3, 128, 128), dtype=BF)
    ones3[0, p, 32 * (p // 32)] = 1.0                        # u rows
    ones3[1, p, 32 * (p // 64) + 1 + (p // 8) % 8] = 1.0     # a, b in {0,1}
    ones3[2, p, 32 * (2 + p // 64) + 1 + (p // 8) % 8] = 1.0  # a, b in {2,3}

    toks = [subjects, relations, objects]
    uniqs, streams, buniqs, idxuas = [], [], [], []
    nu_max = 0
    for core in range(NCORES):
        sl = slice(core * BL, (core + 1) * BL)
        # stream order: b, tau, chunk, m_local, w
        allt = np.stack([t[sl] for t in toks], axis=1)  # [BL, 3, 512, 8]
        uniq, inv = np.unique(allt.reshape(-1), return_inverse=True)
        if len(uniq) > 32752:
            raise OverflowError(f"core {core}: {len(uniq)} unique tokens")
        uniqs.append(uniq)
        streams.append(inv.astype(np.int64))
        nu_max = max(nu_max, len(uniq))
        # b-table side
        bt = np.concatenate([ques[sl].reshape(-1),
                             answerChoices[sl].reshape(-1)])
        bu, binv = np.unique(bt, return_inverse=True)
        assert len(bu) <= BU
        buniqs.append(bu)
        qinv = binv[:BL * QLEN].reshape(BL, QLEN)
        ainv = binv[BL * QLEN:].reshape(BL, NCH, CLEN)
        idxua = np.zeros((128, 3), dtype=np.int32)
        idxua[:, 0] = qinv[p // 32, p % 32]
        idxua[:, 1] = ainv[p // 64, (p // 8) % 8, p % 8]
        idxua[:, 2] = ainv[2 + p // 64, (p // 8) % 8, p % 8]
        idxuas.append(idxua)
    nu_pad = -(-nu_max // 16) * 16

    nc = _build_program(nu_pad)

    a1_bf = A_tables[1].astype(BF)
    a2_bf = A_tables[2].astype(BF)
    if A0FP8:
        a0_q = (A_tables[0] * A0SCALE).astype(F8)
    else:
        a0_q = A_tables[0].astype(BF)

    in_maps = []
    for core in range(NCORES):
        u = uniqs[core]
        tab = np.zeros((nu_pad, ROWE), dtype=BF)
        tab[:len(u), 0:256] = a1_bf[u]
        tab[:len(u), 256:512] = a2_bf[u]
        if A0FP8:
            tab[:len(u), 512:640] = a0_q[u].view(np.uint8).reshape(
                len(u), 256).view(BF)
        else:
            tab[:len(u), 512:768] = a0_q[u]
        btab = np.zeros((BU, EMB), dtype=BF)
        btab[:len(buniqs[core])] = b_bf[buniqs[core]]
        idx16 = np.zeros((128, 3 * BL, NCHUNK * 64), dtype=np.int16)
        stream = streams[core].reshape(BL, 3, NCHUNK, 1024)
        for b in range(BL):
            for tau in range(3):
                for c in range(NCHUNK):
                    idx16[:, b * 3 + tau, 64 * c:64 * c + 64] = \
                        _wrap_idx16(stream[b, tau, c])
        in_maps.append(dict(
            tab=tab, btab=btab, ut=ut, vt=vt, w=w_bf, idx16=idx16,
            selb=selb, self8=self8, ones3=ones3, maskq=maskq,
            identb=identb, crepsel=crepsel, idxua=idxuas[core]))
    return nc, in_maps


def kernel(subjects, relations, objects, ques, answerChoices,
           A_tables, B_table, U, V, W):
    nc, in_maps = _prepare(subjects, relations, objects, ques, answerChoices,
                           A_tables, B_table, U, V, W)
    res = run_bass_kernel_spmd(nc, in_maps, list(range(NCORES)))
    return np.concatenate([res.results[c]["pred"] for c in range(NCORES)],
                          axis=0).astype(np.float32)


def profile(subjects, relations, objects, ques, answerChoices,
            A_tables, B_table, U, V, W, tmpdir=None):
    import os, tempfile
    if tmpdir is None:
        tmpdir = tempfile.mkdtemp(prefix="ktrace_")
    os.makedirs(tmpdir, exist_ok=True)
    nc, in_maps = _prepare(subjects, relations, objects, ques, answerChoices,
                           A_tables, B_table, U, V, W)
    res = run_bass_kernel_spmd(nc, in_maps, list(range(NCORES)),
                               trace=True, tmpdir=tmpdir)
    print(f"trace dir: {tmpdir}")
    return res.exec_time_ns


# revision 35
# speedup vs baseline: 1.1334x; 1.1334x over previous
"""Trainium2 Bass kernel for the 2-hop key-value memory network.

Strategy: data-parallel over batch (B=32 -> 4 per core x 8 cores).

The gather descriptor generation on the GPSIMD (SWDGE Q7) engine is the
hard bottleneck (~8.4 ns per gathered row, serialized), so this version
minimizes gathered rows and hides all other work under the gather stream:

  - Host builds a PER-CORE dense table: the ~31k unique tokens a core
    touches, remapped to dense ids < 32768.  This kills the int16 lo/hi
    index split and its ~25% slot padding: exactly 48 gathers x 1024 rows
    per core, streamed in natural (mem, word) order.
  - Natural order makes the word-sum selection matrix a fixed constant
    (slot s -> mem s//8), so no per-call selection-matrix DMA.
  - Loops are ordered b-outer so each local batch's hop math (scores,
    renorms, weighted sums, final bilinear form) runs under the next
    batch's gathers; only the last batch's hop chain is exposed.

The log-softmax renorms reduce to per-row affine transforms:
  renorm_q: p = (s - min(s)) / (sum(s) - 512*min(s) + 512e-8)
  renorm_a: p = (s - lse)   / (512*lse - sum(s)),  lse = logsumexp(s)
"""
import sys

for _p in ("/opt/pypackages", "/opt/trn_rl_repo"):
    if _p not in sys.path:
        sys.path.insert(0, _p)

import numpy as np
import ml_dtypes

import concourse.bass as bass
import concourse.bacc as bacc
import concourse.mybir as mybir
import concourse.tile as tile
from concourse.bass_utils import run_bass_kernel_spmd

BF = ml_dtypes.bfloat16

# problem constants
B, NMEM, NW, QLEN, NCH, CLEN = 32, 512, 8, 32, 8, 8
VOCAB, M = 50000, 256
EMB = 3 * M           # 768
NCORES = 8
BL = B // NCORES      # 4 batch per core
NCHUNK = 4            # mem chunks of 128 per (b, tau)
BU = 384              # b-table unique slots per core

_cache = {}


def _wrap_idx16(stream):
    """stream (len multiple of 16) -> [128, len/16] int16 wrapped layout:
    index i lives at [i % 16, i // 16], replicated across the 8 groups of 16
    partitions."""
    c = len(stream) // 16
    arr = stream.astype(np.uint16).reshape(c, 16).T
    return np.tile(arr, (8, 1)).view(np.int16)


def _poly_lse(nc, hsb, se, mx, rows):
    """lse = ln(se) + mx via DVE bit-trick cubic -- avoids the Exp<->Ln
    activation-table thrash (natural_log and exp live in different sets)."""
    f32 = mybir.dt.float32
    t = lambda tag: hsb.tile([128, 1], f32, tag=tag, name=tag)
    LN2 = 0.6931471805599453
    C3, C2, C1 = 0.10668396, -0.71358544, 2.08687084
    C0 = -1.47904405 - 127.0 * LN2
    sei = se[rows].bitcast(mybir.dt.int32)
    mb = hsb.tile([128, 1], mybir.dt.int32, tag="mb", name="mb")
    nc.vector.tensor_scalar(out=mb[rows], in0=sei,
                            scalar1=0x007FFFFF, scalar2=0x3F800000,
                            op0=mybir.AluOpType.bitwise_and,
                            op1=mybir.AluOpType.bitwise_or)
    mf = mb[rows].bitcast(f32)
    efi = hsb.tile([128, 1], mybir.dt.int32, tag="efi", name="efi")
    nc.vector.tensor_scalar(out=efi[rows], in0=sei, scalar1=23, scalar2=None,
                            op0=mybir.AluOpType.logical_shift_right)
    ef = t("ef")
    nc.vector.tensor_scalar(out=ef[rows], in0=efi[rows], scalar1=LN2,
                            scalar2=None, op0=mybir.AluOpType.mult)
    lse = t("lse")
    nc.vector.tensor_scalar(out=lse[rows], in0=mf, scalar1=C3, scalar2=C2,
                            op0=mybir.AluOpType.mult,
                            op1=mybir.AluOpType.add)
    nc.vector.tensor_tensor(out=lse[rows], in0=lse[rows], in1=mf,
                            op=mybir.AluOpType.mult)
    nc.vector.tensor_scalar(out=lse[rows], in0=lse[rows], scalar1=C1,
                            scalar2=None, op0=mybir.AluOpType.add)
    nc.vector.tensor_tensor(out=lse[rows], in0=lse[rows], in1=mf,
                            op=mybir.AluOpType.mult)
    nc.vector.tensor_scalar(out=lse[rows], in0=lse[rows], scalar1=C0,
                            scalar2=None, op0=mybir.AluOpType.add)
    nc.vector.tensor_tensor(out=lse[rows], in0=lse[rows], in1=ef[rows],
                            op=mybir.AluOpType.add)
    nc.vector.tensor_tensor(out=lse[rows], in0=lse[rows], in1=mx[rows],
                            op=mybir.AluOpType.add)
    return lse


def _renorm_rows(nc, hsb, ppool, S, rows, maskq, hop, b):
    """Affine renorm of 9 score rows: P = (S - A) / B with
    q-row (p%32==0): A=min, B=sum-512*min+512e-8
    a-rows:          A=lse, B=512*lse-sum."""
    f32 = mybir.dt.float32
    bf16 = mybir.dt.bfloat16
    t = lambda tag: hsb.tile([128, 1], f32, tag=tag, name=tag)
    mx, mn, sm = t("mx"), t("mn"), t("sm")
    nc.vector.tensor_reduce(out=mx[rows], in_=S[rows],
                            axis=mybir.AxisListType.X, op=mybir.AluOpType.max)
    nc.vector.tensor_reduce(out=mn[rows], in_=S[rows],
                            axis=mybir.AxisListType.X, op=mybir.AluOpType.min)
    nc.vector.tensor_reduce(out=sm[rows], in_=S[rows],
                            axis=mybir.AxisListType.X, op=mybir.AluOpType.add)
    negmx = t("negmx")
    nc.vector.tensor_scalar(out=negmx[rows], in0=mx[rows], scalar1=-1.0,
                            scalar2=None, op0=mybir.AluOpType.mult)
    texp = ppool.tile([128, NMEM], f32, tag="texp")
    se = t("se")
    nc.scalar.activation(out=texp[rows], in_=S[rows],
                         func=mybir.ActivationFunctionType.Exp,
                         bias=negmx[rows], scale=1.0, accum_out=se[rows])
    lse = _poly_lse(nc, hsb, se, mx, rows)
    Av, invb = _renorm_consts(nc, hsb, mx, mn, sm, lse, maskq, rows)
    P = ppool.tile([128, NMEM], bf16, tag="P")
    nc.vector.tensor_scalar(out=P[rows], in0=S[rows], scalar1=Av[rows],
                            scalar2=invb[rows], op0=mybir.AluOpType.subtract,
                            op1=mybir.AluOpType.mult)
    return P


def _renorm_consts(nc, hsb, mx, mn, sm, lse, maskq, rows):
    """A = blend(mn, lse); 1/B = 1/blend(sum-512*min+512e-8, 512*lse-sum)."""
    f32 = mybir.dt.float32
    t = lambda tag: hsb.tile([128, 1], f32, tag=tag, name=tag)
    t1 = t("t1")
    nc.vector.tensor_tensor(out=t1[rows], in0=mn[rows], in1=lse[rows],
                            op=mybir.AluOpType.subtract)
    nc.vector.tensor_tensor(out=t1[rows], in0=t1[rows], in1=maskq[rows],
                            op=mybir.AluOpType.mult)
    Av = t("Av")
    nc.vector.tensor_tensor(out=Av[rows], in0=lse[rows], in1=t1[rows],
                            op=mybir.AluOpType.add)
    bq = t("bq")
    nc.vector.tensor_scalar(out=bq[rows], in0=mn[rows], scalar1=-512.0,
                            scalar2=512e-8, op0=mybir.AluOpType.mult,
                            op1=mybir.AluOpType.add)
    nc.vector.tensor_tensor(out=bq[rows], in0=bq[rows], in1=sm[rows],
                            op=mybir.AluOpType.add)
    ba = t("ba")
    nc.vector.tensor_scalar(out=ba[rows], in0=lse[rows], scalar1=512.0,
                            scalar2=None, op0=mybir.AluOpType.mult)
    nc.vector.tensor_tensor(out=ba[rows], in0=ba[rows], in1=sm[rows],
                            op=mybir.AluOpType.subtract)
    Bv = t("Bv")
    nc.vector.tensor_tensor(out=Bv[rows], in0=bq[rows], in1=ba[rows],
                            op=mybir.AluOpType.subtract)
    nc.vector.tensor_tensor(out=Bv[rows], in0=Bv[rows], in1=maskq[rows],
                            op=mybir.AluOpType.mult)
    nc.vector.tensor_tensor(out=Bv[rows], in0=Bv[rows], in1=ba[rows],
                            op=mybir.AluOpType.add)
    invb = t("invb")
    nc.vector.reciprocal(out=invb[rows], in_=Bv[rows])
    return Av, invb


def _build_program(nu_pad):
    """Build + compile the SPMD program (same for all cores)."""
    if nu_pad in _cache:
        return _cache[nu_pad]

    f32 = mybir.dt.float32
    bf16 = mybir.dt.bfloat16

    nc = bacc.Bacc("TRN2", target_bir_lowering=False, debug=False,
                   num_swdge_queues=4,
                   dynamic_dma_scratch_size=24576)
    tab_d = nc.dram_tensor("tab", [nu_pad, EMB], bf16, kind="ExternalInput")
    btab_d = nc.dram_tensor("btab", [BU, EMB], bf16, kind="ExternalInput")
    ut_d = nc.dram_tensor("ut", [EMB, EMB], bf16, kind="ExternalInput")
    vt_d = nc.dram_tensor("vt", [EMB, EMB], bf16, kind="ExternalInput")
    w_d = nc.dram_tensor("w", [EMB, EMB], bf16, kind="ExternalInput")
    idx16_d = nc.dram_tensor("idx16", [128, 3 * BL, NCHUNK * 64],
                             mybir.dt.int16, kind="ExternalInput")
    sel_d = nc.dram_tensor("sel", [128, 8 * 128], bf16, kind="ExternalInput")
    ones3_d = nc.dram_tensor("ones3", [3, 128, 128], bf16,
                             kind="ExternalInput")
    maskq_d = nc.dram_tensor("maskq", [128, 1], f32, kind="ExternalInput")
    identb_d = nc.dram_tensor("identb", [128, 128], bf16,
                              kind="ExternalInput")
    identf_d = nc.dram_tensor("identf", [128, 128], f32,
                              kind="ExternalInput")
    crepsel_d = nc.dram_tensor("crepsel", [128, 16], bf16,
                               kind="ExternalInput")
    idxua_d = nc.dram_tensor("idxua", [128, 3], mybir.dt.int32,
                             kind="ExternalInput")
    out_d = nc.dram_tensor("pred", [BL, NCH], f32, kind="ExternalOutput")

    with tile.TileContext(nc) as tc:
        with (
            tc.tile_pool(name="const", bufs=1) as constp,
            tc.tile_pool(name="state", bufs=1) as statep,
            tc.tile_pool(name="gp", bufs=7) as gp,
            tc.tile_pool(name="ep", bufs=2) as ep,
            tc.tile_pool(name="stgp", bufs=2) as stgp,
            tc.tile_pool(name="hsb", bufs=1) as hsb,
            tc.tile_pool(name="ppool", bufs=2) as ppool,
            tc.tile_pool(name="wsps", bufs=1, space="PSUM") as wsps,
            tc.tile_pool(name="tpps", bufs=2, space="PSUM") as tpps,
            tc.tile_pool(name="hps", bufs=1, space="PSUM") as hps,
            tc.tile_pool(name="hps2", bufs=2, space="PSUM") as hps2,
        ):
            # ---------------- constants ----------------
            idx_sb = constp.tile([128, 3 * BL, NCHUNK * 64], mybir.dt.int16,
                                 tag="idx")
            # first batch's slice lands fast so gather #0 starts early
            nc.sync.dma_start(out=idx_sb[:, 0:3, :], in_=idx16_d[:, 0:3, :])
            nc.sync.dma_start(out=idx_sb[:, 3:3 * BL, :],
                              in_=idx16_d[:, 3:3 * BL, :])
            sel_sb = constp.tile([128, 8 * 128], bf16, tag="sel")
            nc.sync.dma_start(out=sel_sb[:], in_=sel_d[:])
            identb = constp.tile([128, 128], bf16, tag="identb")
            nc.sync.dma_start(out=identb[:], in_=identb_d[:])
            identf = constp.tile([128, 128], f32, tag="identf")
            nc.sync.dma_start(out=identf[:], in_=identf_d[:])
            maskq = constp.tile([128, 1], f32, tag="maskq")
            nc.sync.dma_start(out=maskq[:], in_=maskq_d[:])
            ones3 = [constp.tile([128, 128], bf16, tag=f"ones{i}",
                                 name=f"ones{i}") for i in range(3)]
            for i in range(3):
                nc.sync.dma_start(out=ones3[i][:], in_=ones3_d[i])
            crepsel = constp.tile([128, 16], bf16, tag="crepsel")
            nc.sync.dma_start(out=crepsel[:], in_=crepsel_d[:])
            idxua = constp.tile([128, 3], mybir.dt.int32, tag="idxua")
            nc.sync.dma_start(out=idxua[:], in_=idxua_d[:])
            ut_sb = constp.tile([128, 6, EMB], bf16, tag="ut")
            vt_sb = constp.tile([128, 6, EMB], bf16, tag="vt")
            w_sb = constp.tile([128, 6, EMB], bf16, tag="w")

            def emit_uvw_dma():
                for t_sb, t_d in ((ut_sb, ut_d), (vt_sb, vt_d),
                                  (w_sb, w_d)):
                    nc.sync.dma_start(
                        out=t_sb[:],
                        in_=t_d[:].rearrange("(j p) d -> p j d", p=128))

            # persistent state
            X = statep.tile([128, EMB], f32, tag="X")
            o_sb = statep.tile([128, EMB], bf16, tag="o_sb")
            xtq = statep.tile([128, 6, BL], bf16, tag="xtq")
            xta = statep.tile([128, 6, 8 * BL], bf16, tag="xta")
            ysb0 = statep.tile([128, 6, 9 * BL], bf16, tag="ysb0")
            pred_sb = statep.tile([128, NCH], f32, tag="pred_sb")
            gb = statep.tile([128, 3, EMB], bf16, tag="gb")
            # rank-1 rhs staging for the last batch's tail: rows outside the
            # written band must be finite (0 * NaN != 0 on the PE)
            RSBbf = statep.tile([128, EMB], bf16, tag="rsbbf")
            nc.vector.memset(RSBbf[:], 0)

            # warm-up: force the SWDGE gather ucode LOAD_LIB before the
            # idx DMA lands so the first real gather isn't serialized on it
            warm_idx = constp.tile([128, 1], mybir.dt.int16, tag="warm_idx")
            nc.vector.memset(warm_idx[:], 0)
            warm_g = constp.tile([128, 1, EMB], bf16, tag="warm_g")
            nc.gpsimd.dma_gather(warm_g[:], btab_d[:], warm_idx[:],
                                 16, 16, EMB)

            # ---------------- init: u and a from B_table ----------------
            # emitted inside the b-loop (after the first chunk's gathers) so
            # the gather stream starts immediately
            def emit_init():
                for i in range(3):
                    nc.gpsimd.indirect_dma_start(
                        out=gb[:, i, :], out_offset=None, in_=btab_d[:],
                        in_offset=bass.IndirectOffsetOnAxis(
                            ap=idxua[:, i:i + 1], axis=0))
                psi = hps.tile([128, EMB], f32, tag="O", name="psi")
                for i in range(3):
                    nc.tensor.matmul(out=psi[:, 0:512], lhsT=ones3[i][:],
                                     rhs=gb[:, i, 0:512],
                                     start=(i == 0), stop=(i == 2))
                for i in range(3):
                    nc.tensor.matmul(out=psi[:, 512:768], lhsT=ones3[i][:],
                                     rhs=gb[:, i, 512:768],
                                     start=(i == 0), stop=(i == 2))
                nc.vector.tensor_copy(out=X[:, 0:512], in_=psi[:, 0:512])
                nc.vector.tensor_copy(out=X[:, 512:768], in_=psi[:, 512:768])
                for j in range(6):
                    tp = tpps.tile([128, 128], f32, tag="tp", name="tpi")
                    nc.tensor.transpose(out=tp[:],
                                        in_=X[:, 128 * j:128 * j + 128],
                                        identity=identf[:])
                    tpv = tp[:].rearrange("p (b n) -> p b n", b=BL)
                    nc.any.tensor_copy(out=xtq[:, j, :], in_=tpv[:, :, 0])
                    nc.any.tensor_copy(out=xta[:, j, :], in_=tpv[:, :, 1:9])
                for i in range(6):
                    y0 = hps.tile([128, EMB], f32, tag="O", name="y0")
                    for j in range(6):
                        nc.tensor.matmul(
                            out=y0[:, 0:BL],
                            lhsT=ut_sb[:, j, 128 * i:128 * i + 128],
                            rhs=xtq[:, j, :], start=(j == 0), stop=(j == 5))
                    for j in range(6):
                        nc.tensor.matmul(
                            out=y0[:, BL:9 * BL],
                            lhsT=vt_sb[:, j, 128 * i:128 * i + 128],
                            rhs=xta[:, j, :], start=(j == 0), stop=(j == 5))
                    y9 = ysb0[:, i, :].rearrange("p (b r) -> p b r", b=BL)
                    ya4 = y0[:, BL:9 * BL].rearrange("p (b r) -> p b r", b=BL)
                    nc.any.tensor_copy(out=y9[:, :, 0], in_=y0[:, 0:BL])
                    nc.any.tensor_copy(out=y9[:, :, 1:9], in_=ya4[:])

            # ---------------- per-batch pipeline ----------------
            # hop math is emitted in segments interleaved between the NEXT
            # batch's chunks so PE waits (renorms) always overlap queued
            # word-sum matmuls.
            def hop_chain(b, S0, E1b, E2b, E0Tb, E1Tb):
                rows = slice(32 * b, 32 * b + 9)
                # --- segment A: hop0 scores (batched) + renorm ---
                for j in range(6):
                    nc.tensor.matmul(
                        out=S0[rows, :],
                        lhsT=ysb0[:, j, 9 * b:9 * b + 9],
                        rhs=E0Tb[:, j, :], start=(j == 0), stop=(j == 5),
                        tile_position=(0, 32 * b))
                P0 = _renorm_rows(nc, hsb, ppool, S0[:], rows, maskq, 0, b)
                yield
                # --- segment B: hop0 weighted sum + state update ---
                PT = hsb.tile([128, NCHUNK, 16], bf16, tag=f"PT0_{b}",
                              name="PT")
                for k in range(NCHUNK):
                    tp = tpps.tile([128, 128], bf16, tag="tp", name="tp")
                    nc.tensor.transpose(
                        out=tp[:], in_=P0[:, 128 * k:128 * k + 128],
                        identity=identb[:])
                    nc.any.tensor_copy(out=PT[:, k, 0:9],
                                       in_=tp[:, 32 * b:32 * b + 9])
                O0 = hps.tile([128, EMB], f32, tag="O", name="O0")
                for k in range(NCHUNK):
                    nc.tensor.matmul(
                        out=O0[rows, 0:512], lhsT=PT[:, k, 0:9],
                        rhs=E1b[:, k, 0:512],
                        start=(k == 0), stop=(k == NCHUNK - 1),
                        tile_position=(0, 32 * b))
                    nc.tensor.matmul(
                        out=O0[rows, 512:768], lhsT=PT[:, k, 0:9],
                        rhs=E1b[:, k, 512:768],
                        start=(k == 0), stop=(k == NCHUNK - 1),
                        tile_position=(0, 32 * b))
                yield
                nc.vector.tensor_tensor(out=X[rows, 0:512],
                                        in0=X[rows, 0:512],
                                        in1=O0[rows, 0:512],
                                        op=mybir.AluOpType.add)
                nc.vector.tensor_tensor(out=X[rows, 512:768],
                                        in0=X[rows, 512:768],
                                        in1=O0[rows, 512:768],
                                        op=mybir.AluOpType.add)
                xt1 = hsb.tile([128, 6, 16], bf16, tag=f"xt1_{b}",
                               name="xt1")
                for j in range(6):
                    tp = tpps.tile([128, 128], f32, tag="tp", name="tp")
                    nc.tensor.transpose(
                        out=tp[:], in_=X[:, 128 * j:128 * j + 128],
                        identity=identf[:])
                    nc.any.tensor_copy(out=xt1[:, j, 0:9],
                                       in_=tp[:, 32 * b:32 * b + 9])
                yield
                # --- segment C: hop1 projections + scores + renorm ---
                ysb1 = hsb.tile([128, 6, 9], bf16, tag=f"ysb1_{b}",
                                name="ysb1")
                for i in range(6):
                    if i == 3:
                        yield
                    y1 = hps.tile([128, EMB], f32, tag="O", name="y1")
                    for j in range(6):
                        nc.tensor.matmul(
                            out=y1[:, 0:1],
                            lhsT=ut_sb[:, j, 128 * i:128 * i + 128],
                            rhs=xt1[:, j, 0:1], start=(j == 0),
                            stop=(j == 5))
                    for j in range(6):
                        nc.tensor.matmul(
                            out=y1[:, 1:9],
                            lhsT=vt_sb[:, j, 128 * i:128 * i + 128],
                            rhs=xt1[:, j, 1:9], start=(j == 0),
                            stop=(j == 5))
                    nc.any.tensor_copy(out=ysb1[:, i, :], in_=y1[:, 0:9])
                yield
                S1 = hps2.tile([128, NMEM], f32, tag="S", name="S1")
                for j in range(6):
                    nc.tensor.matmul(
                        out=S1[rows, :], lhsT=ysb1[:, j, 0:9],
                        rhs=E1Tb[:, j, :], start=(j == 0), stop=(j == 5),
                        tile_position=(0, 32 * b))
                P1 = _renorm_rows(nc, hsb, ppool, S1[:], rows, maskq, 1, b)
                yield
                # --- segment D: hop1 weighted sum + final bilinear form ---
                PT1 = hsb.tile([128, NCHUNK, 16], bf16, tag=f"PT1_{b}",
                               name="PT1")
                for k in range(NCHUNK):
                    tp = tpps.tile([128, 128], bf16, tag="tp", name="tp")
                    nc.tensor.transpose(
                        out=tp[:], in_=P1[:, 128 * k:128 * k + 128],
                        identity=identb[:])
                    nc.any.tensor_copy(out=PT1[:, k, 0:9],
                                       in_=tp[:, 32 * b:32 * b + 9])
                O1 = hps.tile([128, EMB], f32, tag="O", name="O1")
                for k in range(NCHUNK):
                    nc.tensor.matmul(
                        out=O1[rows, 0:512], lhsT=PT1[:, k, 0:9],
                        rhs=E2b[:, k, 0:512],
                        start=(k == 0), stop=(k == NCHUNK - 1),
                        tile_position=(0, 32 * b))
                    nc.tensor.matmul(
                        out=O1[rows, 512:768], lhsT=PT1[:, k, 0:9],
                        rhs=E2b[:, k, 512:768],
                        start=(k == 0), stop=(k == NCHUNK - 1),
                        tile_position=(0, 32 * b))
                yield
                nc.any.tensor_copy(out=o_sb[rows, 0:512],
                                   in_=O1[rows, 0:512])
                nc.any.tensor_copy(out=o_sb[rows, 512:768],
                                   in_=O1[rows, 512:768])
                ot = hsb.tile([128, 6, 16], bf16, tag=f"ot_{b}", name="ot")
                for j in range(6):
                    tp = tpps.tile([128, 128], bf16, tag="tp", name="tp")
                    nc.tensor.transpose(
                        out=tp[:], in_=o_sb[:, 128 * j:128 * j + 128],
                        identity=identb[:])
                    nc.any.tensor_copy(out=ot[:, j, 0:9],
                                       in_=tp[:, 32 * b:32 * b + 9])
                yield
                wq = hsb.tile([128, 6, 1], bf16, tag=f"wq_{b}", name="wq")
                for i in range(6):
                    wqp = hps.tile([128, EMB], f32, tag="O", name="wqp")
                    for j in range(6):
                        nc.tensor.matmul(
                            out=wqp[:, 0:1],
                            lhsT=w_sb[:, j, 128 * i:128 * i + 128],
                            rhs=ot[:, j, 0:1], start=(j == 0), stop=(j == 5))
                    nc.any.tensor_copy(out=wq[:, i, :], in_=wqp[:, 0:1])
                predp = hps.tile([128, EMB], f32, tag="O", name="predp")
                for i in range(6):
                    nc.tensor.matmul(
                        out=predp[32 * b:32 * b + 1, 0:NCH],
                        lhsT=wq[:, i, 0:1], rhs=ot[:, i, 1:9],
                        start=(i == 0), stop=(i == 5),
                        tile_position=(0, 32 * b))
                nc.vector.tensor_copy(out=pred_sb[32 * b:32 * b + 1, :],
                                      in_=predp[32 * b:32 * b + 1, 0:NCH])
                nc.sync.dma_start(out=out_d[b:b + 1, :],
                                  in_=pred_sb[32 * b:32 * b + 1, :])
                yield

            pending = None
            for b in range(BL):
                last = (b == BL - 1)
                E1b = ep.tile([128, NCHUNK, EMB], bf16, tag="E1", name="E1b")
                E2b = ep.tile([128, NCHUNK, EMB], bf16, tag="E2", name="E2b")
                E0Tb = ep.tile([128, 6, NMEM], bf16, tag="E0T", name="E0Tb")
                E1Tb = ep.tile([128, 6, NMEM], bf16, tag="E1T", name="E1Tb")
                S0 = hps2.tile([128, NMEM], f32, tag="S", name="S0")
                if last:
                    rows = slice(32 * b, 32 * b + 9)
                    rows10 = slice(32 * b, 32 * b + 10)
                    prt0 = hsb.tile([128, 12], f32, tag="prt0", name="prt0")
                    Ssb0 = hsb.tile([128, NMEM], bf16, tag="ssb0t",
                                    name="Ssb0")
                    stage0 = hsb.tile([128, NCHUNK, 16], bf16, tag="stage0t",
                                      name="stage0")
                    stage1 = hsb.tile([128, NCHUNK, 16], bf16, tag="stage1t",
                                      name="stage1")
                    xta1 = hsb.tile([128, 6, 9], bf16, tag="xta1t",
                                    name="xta1")
                    nc.vector.memset(stage0[:, :, 9:10], 1.0)
                    nc.vector.memset(stage1[:, :, 9:10], 1.0)
                    nc.vector.memset(xta1[:, :, 0:1], 0)
                for c in range(NCHUNK):
                    for tau in range(3):
                        tb = b * 3 + tau
                        g = gp.tile([128, 8, EMB], bf16, tag="g", name="g")
                        nc.gpsimd.dma_gather(
                            g[:], tab_d[:],
                            idx_sb[:, tb, 64 * c:64 * c + 64],
                            1024, 1024, EMB,
                            queue_num=(c * 3 + tau) % 4)
                        ps_a = wsps.tile([128, 512], f32, tag="wsa",
                                         name="ps_a")
                        ps_b = wsps.tile([128, 256], f32, tag="wsb",
                                         name="ps_b")
                        for gi in range(8):
                            k = gi // 2
                            lhs = sel_sb[:, 128 * gi + 32 * k:
                                         128 * gi + 32 * k + 32]
                            nc.tensor.matmul(
                                out=ps_a[32 * k:32 * k + 32, :], lhsT=lhs,
                                rhs=g[:, gi, 0:512],
                                start=(gi % 2 == 0), stop=(gi % 2 == 1),
                                tile_position=(0, 32 * k))
                            nc.tensor.matmul(
                                out=ps_b[32 * k:32 * k + 32, :], lhsT=lhs,
                                rhs=g[:, gi, 512:768],
                                start=(gi % 2 == 0), stop=(gi % 2 == 1),
                                tile_position=(0, 32 * k))
                        dl = 256 * tau
                        nc.any.tensor_copy(out=E1b[:, c, dl:dl + 256],
                                           in_=ps_a[:, 256:512])
                        nc.any.tensor_copy(out=E2b[:, c, dl:dl + 256],
                                           in_=ps_b[:])
                        stg = stgp.tile([128, 512], bf16, tag="stg",
                                        name="stg")
                        nc.any.tensor_copy(out=stg[:], in_=ps_a[:])
                        for q in range(4):  # 0,1 -> E0T ; 2,3 -> E1T
                            tp = tpps.tile([128, 128], bf16, tag="tp",
                                           name="tp")
                            nc.tensor.transpose(
                                out=tp[:], in_=stg[:, 128 * q:128 * q + 128],
                                identity=identb[:])
                            dst = E0Tb if q < 2 else E1Tb
                            nc.any.tensor_copy(
                                out=dst[:, 2 * tau + (q % 2),
                                        128 * c:128 * c + 128],
                                in_=tp[:])
                        if pending is not None and tau < 2:
                            next(pending, None)
                    if b == 0 and c == 0:
                        emit_uvw_dma()
                        emit_init()
                    if last:
                        # chunk-level hop-0 scores + renorm partials so the
                        # exposed tail only holds the combine
                        mc = slice(128 * c, 128 * c + 128)
                        for j in range(6):
                            nc.tensor.matmul(
                                out=S0[rows, mc],
                                lhsT=ysb0[:, j, 9 * b:9 * b + 9],
                                rhs=E0Tb[:, j, mc],
                                start=(j == 0), stop=(j == 5),
                                tile_position=(0, 32 * b))
                        nc.any.tensor_copy(out=Ssb0[rows, mc],
                                           in_=S0[rows, mc])
                        for col, op in ((c, mybir.AluOpType.max),
                                        (4 + c, mybir.AluOpType.min),
                                        (8 + c, mybir.AluOpType.add)):
                            nc.vector.tensor_reduce(
                                out=prt0[rows, col:col + 1],
                                in_=S0[rows, mc],
                                axis=mybir.AxisListType.X, op=op)
                    if pending is not None:
                        next(pending, None)
                if not last:
                    pending = hop_chain(b, S0, E1b, E2b, E0Tb, E1Tb)
                    continue
                # ------------- last batch's exposed tail -------------
                if pending is not None:
                    for _ in pending:
                        pass
                    pending = None
                t1h = lambda tag: hsb.tile([128, 1], f32, tag=tag, name=tag)
                # hop-0 renorm constants (DVE/scalar) -- emitted first so
                # they run in parallel with the PE staging below
                mx, mn, sm = t1h("mx"), t1h("mn"), t1h("sm")
                nc.vector.tensor_reduce(out=mx[rows], in_=prt0[rows, 0:4],
                                        axis=mybir.AxisListType.X,
                                        op=mybir.AluOpType.max)
                nc.vector.tensor_reduce(out=mn[rows], in_=prt0[rows, 4:8],
                                        axis=mybir.AxisListType.X,
                                        op=mybir.AluOpType.min)
                nc.vector.tensor_reduce(out=sm[rows], in_=prt0[rows, 8:12],
                                        axis=mybir.AxisListType.X,
                                        op=mybir.AluOpType.add)
                negmx = t1h("negmx")
                nc.vector.tensor_scalar(out=negmx[rows], in0=mx[rows],
                                        scalar1=-1.0, scalar2=None,
                                        op0=mybir.AluOpType.mult)
                texp = ppool.tile([128, NMEM], f32, tag="texp")
                se = t1h("se")
                nc.scalar.activation(out=texp[rows], in_=S0[rows, :],
                                     func=mybir.ActivationFunctionType.Exp,
                                     bias=negmx[rows], scale=1.0,
                                     accum_out=se[rows])
                lse = _poly_lse(nc, hsb, se, mx, rows)
                Av0, invb0 = _renorm_consts(nc, hsb, mx, mn, sm, lse,
                                            maskq, rows)
                # PE: S^T staging + R0 + Crep0, parallel to the renorm
                for c in range(NCHUNK):
                    mc = slice(128 * c, 128 * c + 128)
                    tp = tpps.tile([128, 128], bf16, tag="tp", name="tpS")
                    nc.tensor.transpose(out=tp[:], in_=Ssb0[:, mc],
                                        identity=identb[:])
                    nc.any.tensor_copy(out=stage0[:, c, 0:9],
                                       in_=tp[:, 32 * b:32 * b + 9])
                R0 = hps.tile([128, EMB], f32, tag="O", name="R0")
                for lo, hi in ((0, 512), (512, 768)):
                    for c in range(NCHUNK):
                        nc.tensor.matmul(out=R0[rows10, lo:hi],
                                         lhsT=stage0[:, c, 0:10],
                                         rhs=E1b[:, c, lo:hi],
                                         start=(c == 0), stop=(c == 3),
                                         tile_position=(0, 32 * b))
                nc.any.tensor_copy(out=RSBbf[rows10, :], in_=R0[rows10, :])
                Crep0 = hps.tile([128, EMB], f32, tag="O", name="Crep0")
                for lo, hi in ((0, 512), (512, 768)):
                    nc.tensor.matmul(out=Crep0[rows, lo:hi],
                                     lhsT=crepsel[:, 0:9],
                                     rhs=RSBbf[:, lo:hi],
                                     start=True, stop=True,
                                     tile_position=(0, 32 * b))
                # O0 = (R0 - A*C)/B; X += O0   (pure DVE)
                otmp = hsb.tile([128, EMB], bf16, tag="otmp", name="otmp")
                nc.vector.tensor_scalar(out=otmp[rows, :],
                                        in0=Crep0[rows, :],
                                        scalar1=Av0[rows], scalar2=None,
                                        op0=mybir.AluOpType.mult)
                nc.vector.tensor_tensor(out=otmp[rows, :],
                                        in0=RSBbf[rows, :],
                                        in1=otmp[rows, :],
                                        op=mybir.AluOpType.subtract)
                nc.vector.tensor_scalar(out=otmp[rows, :], in0=otmp[rows, :],
                                        scalar1=invb0[rows], scalar2=None,
                                        op0=mybir.AluOpType.mult)
                nc.vector.tensor_tensor(out=X[rows, :], in0=X[rows, :],
                                        in1=otmp[rows, :],
                                        op=mybir.AluOpType.add)
                # xt1 staging
                Xbf = hsb.tile([128, EMB], bf16, tag="tbf", name="Xbf")
                nc.any.tensor_copy(out=Xbf[rows, :], in_=X[rows, :])
                xt1u = hsb.tile([128, 6, 1], bf16, tag="xt1ut", name="xt1u")
                for j in range(6):
                    tp = tpps.tile([128, 128], bf16, tag="tp", name="tpX")
                    nc.tensor.transpose(out=tp[:],
                                        in_=Xbf[:, 128 * j:128 * j + 128],
                                        identity=identb[:])
                    nc.any.tensor_copy(out=xt1u[:, j, :],
                                       in_=tp[:, 32 * b:32 * b + 1])
                    nc.any.tensor_copy(out=xta1[:, j, 1:9],
                                       in_=tp[:, 32 * b + 1:32 * b + 9])
                # y1^T then ysb1 (d on partitions)
                y1T = hps.tile([128, EMB], f32, tag="O", name="y1T")
                for lo, hi in ((0, 512), (512, 768)):
                    for j in range(6):
                        nc.tensor.matmul(out=y1T[0:9, lo:hi],
                                         lhsT=xta1[:, j, 0:9],
                                         rhs=vt_sb[:, j, lo:hi],
                                         start=(j == 0), stop=False)
                    for j in range(6):
                        nc.tensor.matmul(out=y1T[0:1, lo:hi],
                                         lhsT=xt1u[:, j, 0:1],
                                         rhs=ut_sb[:, j, lo:hi],
                                         start=False, stop=(j == 5))
                y1Tsb = hsb.tile([128, EMB], bf16, tag="tbf",
                                 name="y1Tsb")
                nc.any.tensor_copy(out=y1Tsb[0:9, :], in_=y1T[0:9, :])
                ysb1 = hsb.tile([128, 6, 9], bf16, tag="ysb1t", name="ysb1")
                for j in range(6):
                    tp = tpps.tile([128, 128], bf16, tag="tp", name="tpY")
                    nc.tensor.transpose(out=tp[:],
                                        in_=y1Tsb[:, 128 * j:128 * j + 128],
                                        identity=identb[:])
                    nc.any.tensor_copy(out=ysb1[:, j, :], in_=tp[:, 0:9])
                # hop-1 scores
                S1 = hps2.tile([128, NMEM], f32, tag="S", name="S1")
                for j in range(6):
                    nc.tensor.matmul(out=S1[rows, :], lhsT=ysb1[:, j, 0:9],
                                     rhs=E1Tb[:, j, :],
                                     start=(j == 0), stop=(j == 5),
                                     tile_position=(0, 32 * b))
                Ssb1 = hsb.tile([128, NMEM], bf16, tag="ssb1t", name="Ssb1")
                nc.any.tensor_copy(out=Ssb1[rows, :], in_=S1[rows, :])
                # renorm1 consts (DVE) emitted first, PE staging after
                mx1, mn1, sm1 = t1h("mx"), t1h("mn"), t1h("sm")
                nc.vector.tensor_reduce(out=mx1[rows], in_=S1[rows, :],
                                        axis=mybir.AxisListType.X,
                                        op=mybir.AluOpType.max)
                nc.vector.tensor_reduce(out=mn1[rows], in_=S1[rows, :],
                                        axis=mybir.AxisListType.X,
                                        op=mybir.AluOpType.min)
                nc.vector.tensor_reduce(out=sm1[rows], in_=S1[rows, :],
                                        axis=mybir.AxisListType.X,
                                        op=mybir.AluOpType.add)
                negmx1 = t1h("negmx")
                nc.vector.tensor_scalar(out=negmx1[rows], in0=mx1[rows],
                                        scalar1=-1.0, scalar2=None,
                                        op0=mybir.AluOpType.mult)
                texp1 = ppool.tile([128, NMEM], f32, tag="texp")
                se1 = t1h("se")
                nc.scalar.activation(out=texp1[rows], in_=S1[rows, :],
                                     func=mybir.ActivationFunctionType.Exp,
                                     bias=negmx1[rows], scale=1.0,
                                     accum_out=se1[rows])
                lse1 = _poly_lse(nc, hsb, se1, mx1, rows)
                Av1, invb1 = _renorm_consts(nc, hsb, mx1, mn1, sm1, lse1,
                                            maskq, rows)
                for c in range(NCHUNK):
                    mc = slice(128 * c, 128 * c + 128)
                    tp = tpps.tile([128, 128], bf16, tag="tp", name="tpS1")
                    nc.tensor.transpose(out=tp[:], in_=Ssb1[:, mc],
                                        identity=identb[:])
                    nc.any.tensor_copy(out=stage1[:, c, 0:9],
                                       in_=tp[:, 32 * b:32 * b + 9])
                R1 = hps.tile([128, EMB], f32, tag="O", name="R1")
                for lo, hi in ((0, 512), (512, 768)):
                    for c in range(NCHUNK):
                        nc.tensor.matmul(out=R1[rows10, lo:hi],
                                         lhsT=stage1[:, c, 0:10],
                                         rhs=E2b[:, c, lo:hi],
                                         start=(c == 0), stop=(c == 3),
                                         tile_position=(0, 32 * b))
                nc.any.tensor_copy(out=RSBbf[rows10, :], in_=R1[rows10, :])
                Crep1 = hps.tile([128, EMB], f32, tag="O", name="Crep1")
                for lo, hi in ((0, 512), (512, 768)):
                    nc.tensor.matmul(out=Crep1[rows, lo:hi],
                                     lhsT=crepsel[:, 0:9],
                                     rhs=RSBbf[:, lo:hi],
                                     start=True, stop=True,
                                     tile_position=(0, 32 * b))
                # O1 -> o_sb
                nc.vector.tensor_scalar(out=otmp[rows, :],
                                        in0=Crep1[rows, :],
                                        scalar1=Av1[rows], scalar2=None,
                                        op0=mybir.AluOpType.mult)
                nc.vector.tensor_tensor(out=otmp[rows, :],
                                        in0=RSBbf[rows, :],
                                        in1=otmp[rows, :],
                                        op=mybir.AluOpType.subtract)
                nc.vector.tensor_scalar(out=o_sb[rows, :],
                                        in0=otmp[rows, :],
                                        scalar1=invb1[rows], scalar2=None,
                                        op0=mybir.AluOpType.mult)
                # final bilinear form via t = o_q @ W (rhs-streamed)
                ot = hsb.tile([128, 6, 16], bf16, tag="ott", name="ot")
                for j in range(6):
                    tp = tpps.tile([128, 128], bf16, tag="tp", name="tpO")
                    nc.tensor.transpose(
                        out=tp[:], in_=o_sb[:, 128 * j:128 * j + 128],
                        identity=identb[:])
                    nc.any.tensor_copy(out=ot[:, j, 0:9],
                                       in_=tp[:, 32 * b:32 * b + 9])
                tvec = hps.tile([128, EMB], f32, tag="O", name="tvec")
                for lo, hi in ((0, 512), (512, 768)):
                    for j in range(6):
                        nc.tensor.matmul(out=tvec[0:1, lo:hi],
                                         lhsT=ot[:, j, 0:1],
                                         rhs=w_sb[:, j, lo:hi],
                                         start=(j == 0), stop=(j == 5))
                tsb = hsb.tile([128, EMB], bf16, tag="tbf", name="tsb")
                nc.any.tensor_copy(out=tsb[0:1, :], in_=tvec[0:1, :])
                tT = hsb.tile([128, 6, 1], bf16, tag="tTt", name="tT")
                for j in range(6):
                    tp = tpps.tile([128, 128], bf16, tag="tp", name="tpT")
                    nc.tensor.transpose(out=tp[:],
                                        in_=tsb[:, 128 * j:128 * j + 128],
                                        identity=identb[:])
                    nc.any.tensor_copy(out=tT[:, j, :], in_=tp[:, 0:1])
                predp = hps.tile([128, EMB], f32, tag="O", name="predp")
                for j in range(6):
                    nc.tensor.matmul(
                        out=predp[32 * b:32 * b + 1, 0:NCH],
                        lhsT=tT[:, j, 0:1], rhs=ot[:, j, 1:9],
                        start=(j == 0), stop=(j == 5),
                        tile_position=(0, 32 * b))
                nc.vector.tensor_copy(out=pred_sb[32 * b:32 * b + 1, :],
                                      in_=predp[32 * b:32 * b + 1, 0:NCH])
                nc.sync.dma_start(out=out_d[b:b + 1, :],
                                  in_=pred_sb[32 * b:32 * b + 1, :])

    nc.compile()
    _cache[nu_pad] = nc
    return nc


def _prepare(subjects, relations, objects, ques, answerChoices,
             A_tables, B_table, U, V, W):
    subjects = np.asarray(subjects).astype(np.int64)
    relations = np.asarray(relations).astype(np.int64)
    objects = np.asarray(objects).astype(np.int64)
    ques = np.asarray(ques).astype(np.int64)
    answerChoices = np.asarray(answerChoices).astype(np.int64)
    A_tables = np.asarray(A_tables, dtype=np.float32)
    B_table = np.asarray(B_table, dtype=np.float32)

    a_cat = np.concatenate([A_tables[0], A_tables[1], A_tables[2]],
                           axis=1).astype(BF)
    b_bf = B_table.astype(BF)
    ut = np.ascontiguousarray(np.asarray(U, dtype=np.float32).T).astype(BF)
    vt = np.ascontiguousarray(np.asarray(V, dtype=np.float32).T).astype(BF)
    w_bf = np.ascontiguousarray(np.asarray(W, dtype=np.float32)).astype(BF)
    identb = np.eye(128, dtype=BF)
    identf = np.eye(128, dtype=np.float32)
    maskq = np.zeros((128, 1), dtype=np.float32)
    maskq[0::32] = 1.0
    # fixed word-sum selection: slot s = gi*128+p -> mem 16*gi + p//8
    p = np.arange(128)
    sel = np.zeros((128, 8, 128), dtype=BF)
    for gi in range(8):
        sel[p, gi, 16 * gi + p // 8] = 1.0
    sel = sel.reshape(128, 8 * 128)
    # rank-1 selector for the last batch's tail: replicate R's C-row
    # (partition 32*(BL-1)+9) into the 9 score rows
    crepsel = np.zeros((128, 16), dtype=BF)
    crepsel[32 * (BL - 1) + 9, 0:9] = 1.0
    # init placement matrices (state row = 32*b + tc)
    ones3 = np.zeros((3, 128, 128), dtype=BF)
    ones3[0, p, 32 * (p // 32)] = 1.0                        # u rows
    ones3[1, p, 32 * (p // 64) + 1 + (p // 8) % 8] = 1.0     # a, b in {0,1}
    ones3[2, p, 32 * (2 + p // 64) + 1 + (p // 8) % 8] = 1.0  # a, b in {2,3}

    toks = [subjects, relations, objects]
    uniqs, streams, buniqs, idxuas = [], [], [], []
    nu_max = 0
    for core in range(NCORES):
        sl = slice(core * BL, (core + 1) * BL)
        # stream order: b, tau, chunk, m_local, w
        allt = np.stack([t[sl] for t in toks], axis=1)  # [BL, 3, 512, 8]
        uniq, inv = np.unique(allt.reshape(-1), return_inverse=True)
        if len(uniq) > 32752:
            raise OverflowError(f"core {core}: {len(uniq)} unique tokens")
        uniqs.append(uniq)
        streams.append(inv.astype(np.int64))
        nu_max = max(nu_max, len(uniq))
        # b-table side
        bt = np.concatenate([ques[sl].reshape(-1),
                             answerChoices[sl].reshape(-1)])
        bu, binv = np.unique(bt, return_inverse=True)
        assert len(bu) <= BU
        buniqs.append(bu)
        qinv = binv[:BL * QLEN].reshape(BL, QLEN)
        ainv = binv[BL * QLEN:].reshape(BL, NCH, CLEN)
        idxua = np.zeros((128, 3), dtype=np.int32)
        idxua[:, 0] = qinv[p // 32, p % 32]
        idxua[:, 1] = ainv[p // 64, (p // 8) % 8, p % 8]
        idxua[:, 2] = ainv[2 + p // 64, (p // 8) % 8, p % 8]
        idxuas.append(idxua)
    nu_pad = -(-nu_max // 16) * 16

    nc = _build_program(nu_pad)

    in_maps = []
    for core in range(NCORES):
        tab = np.zeros((nu_pad, EMB), dtype=BF)
        tab[:len(uniqs[core])] = a_cat[uniqs[core]]
        btab = np.zeros((BU, EMB), dtype=BF)
        btab[:len(buniqs[core])] = b_bf[buniqs[core]]
        idx16 = np.zeros((128, 3 * BL, NCHUNK * 64), dtype=np.int16)
        stream = streams[core].reshape(BL, 3, NCHUNK, 1024)
        for b in range(BL):
            for tau in range(3):
                for c in range(NCHUNK):
                    idx16[:, b * 3 + tau, 64 * c:64 * c + 64] = \
                        _wrap_idx16(stream[b, tau, c])
        in_maps.append(dict(
            tab=tab, btab=btab, ut=ut, vt=vt, w=w_bf, idx16=idx16,
            sel=sel, ones3=ones3, maskq=maskq, identb=identb,
            identf=identf, crepsel=crepsel, idxua=idxuas[core]))
    return nc, in_maps


def kernel(subjects, relations, objects, ques, answerChoices,
           A_tables, B_table, U, V, W):
    nc, in_maps = _prepare(subjects, relations, objects, ques, answerChoices,
                           A_tables, B_table, U, V, W)
    res = run_bass_kernel_spmd(nc, in_maps, list(range(NCORES)))
    return np.concatenate([res.results[c]["pred"] for c in range(NCORES)],
                          axis=0).astype(np.float32)


def profile(subjects, relations, objects, ques, answerChoices,
            A_tables, B_table, U, V, W, tmpdir=None):
    import os, tempfile
    if tmpdir is None:
        tmpdir = tempfile.mkdtemp(prefix="ktrace_")
    os.makedirs(tmpdir, exist_ok=True)
    nc, in_maps = _prepare(subjects, relations, objects, ques, answerChoices,
                           A_tables, B_table, U, V, W)
    res = run_bass_kernel_spmd(nc, in_maps, list(range(NCORES)),
                               trace=True, tmpdir=tmpdir)
    print(f"trace dir: {tmpdir}")
    return res.exec_time_ns

